# revision 1
# baseline (speedup 1.0000x reference)
"""RoFormer self-attention v3: all-bf16 matmuls + DMA transposes +
multi-engine softmax exp, on 8 trn2 cores.

Sharding: core c -> batch b = c//2, head-group g = c%2 (8 of 16 heads).

Per core:
  Phase 1 (per 128-token chunk t, 16 chunks):
    LN stats (DVE bn_stats) -> hn = (ht-mu)*rstd bf16 (DVE tensor_scalar, 2x)
    dmaT hn -> hTb [128, 8, 128] bf16 (lhsT for QKV directly)
    QKV: per output o: ones-row bias matmul + 8 bf16 matmuls [128,128]x[128,512]
    RoPE on PSUM q/k (DVE): col order per head = [even d | odd d] so the
      rotate-half partner is a contiguous 32-col block; q tables carry
      C = log2(e)/8 so scores come out in exp2 units
    dmaT qf/kf -> qT/kT [128, 4, 16, 128] bf16 (partition 64*(h%2)+d')
    v: PSUM -> vA [128, 16, 8, 65] bf16 + ones column
  Phase 2 (per unit u = (head h, query-half qh), 16 units):
    scores: 2 bf16 matmuls lhsT=kT [64,128] rhs=qT [64,512] -> sp [128,1024]
    exp: 2^sp -> U[u%2][:, kc, :] bf16; split ACT Exp(scale=ln2) vs
         chain (DVE copy -> Pool pow(2,x))
    ctx: 16 accumulating matmuls lhsT=vA [128, 65] -> cpd [65, 512] x2
    tail: copy cpd -> bf16 SBUF, DMA out.  Softmax division happens on HOST.
"""

import numpy as np

import concourse.bass as bass
import concourse.mybir as mybir
import concourse.tile as tile
from concourse import bacc
from concourse.bass_utils import run_bass_kernel_spmd

F32 = mybir.dt.float32
BF16 = mybir.dt.bfloat16
AX = mybir.AluOpType
ACT = mybir.ActivationFunctionType

B, S, H = 4, 2048, 1024
NH, HD = 16, 64
LN_EPS = 1e-12
N_CORES = 8
HPC = 8
TOKCH = 16
CEXP = float(np.log2(np.e) / np.sqrt(HD))
LN2 = float(np.log(2.0))

CHAIN_KCS = (1, 4, 7, 10, 13)
import os
FILLERS = int(os.environ.get('FILLERS', '0'))

_CACHE = {}


def _ap(t, offset_elems, free_dims):
    return bass.AP(tensor=t.tensor, offset=t.offset + offset_elems,
                   ap=[[t.ap[0][0], t.ap[0][1]]] + [[s, n] for s, n in free_dims])


def _build_program(phases="12"):
    nc = bacc.Bacc("TRN2", target_bir_lowering=False)

    hid_d = nc.dram_tensor("hid", [S, H], F32, kind="ExternalInput")
    w_d = nc.dram_tensor("w", [128, 8, 1536], BF16, kind="ExternalInput")
    wb_d = nc.dram_tensor("wb", [1, 1536], BF16, kind="ExternalInput")
    onesr_d = nc.dram_tensor("onesr", [1, 128], BF16, kind="ExternalInput")
    cq_d = nc.dram_tensor("cq", [128, TOKCH, 32], BF16, kind="ExternalInput")
    sq_d = nc.dram_tensor("sq", [128, TOKCH, 2, 32], BF16, kind="ExternalInput")
    ck_d = nc.dram_tensor("ck", [128, TOKCH, 32], BF16, kind="ExternalInput")
    sk_d = nc.dram_tensor("sk", [128, TOKCH, 2, 32], BF16, kind="ExternalInput")
    two_d = nc.dram_tensor("two", [128, 2], F32, kind="ExternalInput")
    out_d = nc.dram_tensor("out", [HPC * 2, 2, 65, 512], BF16,
                           kind="ExternalOutput")

    with tile.TileContext(nc) as tc:
        with tc.tile_pool(name="const", bufs=1) as const, \
             tc.tile_pool(name="store", bufs=1) as store:
            w_s = const.tile([128, 8, 1536], BF16)
            wb_s = const.tile([1, 1536], BF16)
            onesr_s = const.tile([1, 128], BF16)
            cq_s = const.tile([128, TOKCH, 32], BF16)
            sq_s = const.tile([128, TOKCH, 2, 32], BF16)
            ck_s = const.tile([128, TOKCH, 32], BF16)
            sk_s = const.tile([128, TOKCH, 2, 32], BF16)
            two_s = const.tile([128, 2], F32)
            nhalf_s = const.tile([128, 1], F32)

            qT = store.tile([128, 4, TOKCH, 128], BF16)
            kT = store.tile([128, 4, TOKCH, 128], BF16)
            vA = store.tile([128, TOKCH, HPC, 65], BF16)
            U0 = store.tile([128, TOKCH, 1024], BF16)
            U1 = store.tile([128, TOKCH, 1024], BF16)
            U = [U0, U1]

            # ---------------- Phase 1 ----------------
            with tc.tile_pool(name="p1h", bufs=4) as p1h, \
                 tc.tile_pool(name="p1s", bufs=3) as p1s, \
                 tc.tile_pool(name="p1n", bufs=2) as p1n, \
                 tc.tile_pool(name="p1t", bufs=3) as p1t, \
                 tc.tile_pool(name="p1r", bufs=4) as p1r, \
                 tc.tile_pool(name="qkvp", bufs=3, space="PSUM") as qkvp, \
                 tc.tile_pool(name="vkvp", bufs=2, space="PSUM") as vkvp:
                nc.sync.dma_start(out=wb_s, in_=wb_d[:, :])
                nc.sync.dma_start(out=onesr_s, in_=onesr_d[:, :])
                nc.gpsimd.memset(nhalf_s, -0.5)
                wu = qkvp.tile([128, 2, 512], F32, tag="qk")
                for _ in range(24):
                    nc.tensor.matmul(wu[:, 0, :], lhsT=onesr_s,
                                     rhs=wb_s[:, 0:512], start=True, stop=True)
                nc.sync.dma_start(out=w_s, in_=w_d[:, :, :])
                nc.sync.dma_start(out=cq_s, in_=cq_d[:, :, :])
                nc.sync.dma_start(out=sq_s, in_=sq_d[:, :, :, :])
                nc.sync.dma_start(out=ck_s, in_=ck_d[:, :, :])
                nc.sync.dma_start(out=sk_s, in_=sk_d[:, :, :, :])
                nc.sync.dma_start(out=two_s, in_=two_d[:, :])

                def copy_block(t, qkt, vt):
                    # q/k PSUM -> SBUF bf16 (ACT) frees PSUM fast; v -> vA
                    qkb = p1r.tile([128, 2, 512], BF16, tag="qkb")
                    nc.scalar.copy(qkb, qkt)
                    nc.scalar.copy(
                        vA[:, t, :, 0:64],
                        vt.rearrange("p (h d) -> p h d", d=64))
                    nc.gpsimd.memset(vA[:, t, :, 64:65], 1.0)
                    return qkb

                def rope_block(t, qkb):
                    for qk in range(2):
                        cos_s = cq_s if qk == 0 else ck_s
                        sin_s = sq_s if qk == 0 else sk_s
                        src = qkb[:, qk, :]
                        qc = p1r.tile([128, 8, 2, 32], BF16, tag=f"qc{qk}")
                        cos_ap = _ap(cos_s, t * 32, [(0, 8), (0, 2), (1, 32)])
                        nc.vector.tensor_tensor(
                            out=qc,
                            in0=src.rearrange("p (h f e) -> p h f e", f=2, e=32),
                            in1=cos_ap, op=AX.mult)
                        sw_ap = _ap(qkb, qk * 512 + 32,
                                    [(64, 8), (-32, 2), (1, 32)])
                        sin_ap = _ap(sin_s, t * 64, [(0, 8), (32, 2), (1, 32)])
                        t2 = p1r.tile([128, 8, 2, 32], BF16, tag=f"t2{qk}")
                        nc.vector.tensor_tensor(out=t2, in0=sw_ap, in1=sin_ap,
                                                op=AX.mult)
                        qf = p1r.tile([128, 512], BF16, tag=f"qf{qk}")
                        nc.vector.tensor_tensor(
                            out=qf.rearrange("p (h f e) -> p h f e", f=2, e=32),
                            in0=qc, in1=t2, op=AX.add)
                        dst = qT if qk == 0 else kT
                        nc.scalar.dma_start_transpose(dst[:, :, t, :], qf)

                pending = []
                pending_c = []
                ht_tiles = {}
                for tpre in range(4):
                    ht = p1h.tile([128, H], F32, tag="ht")
                    nc.scalar.dma_start(out=ht,
                                        in_=hid_d[tpre * 128:(tpre + 1) * 128, :])
                    ht_tiles[tpre] = ht

                for t in range(TOKCH):
                    if t + 4 < TOKCH:
                        htn = p1h.tile([128, H], F32, tag="ht")
                        nc.scalar.dma_start(
                            out=htn, in_=hid_d[(t + 4) * 128:(t + 5) * 128, :])
                        ht_tiles[t + 4] = htn
                    ht = ht_tiles.pop(t)

                    st6 = p1s.tile([128, 2, 6], F32, tag="st6")
                    for hf in range(2):
                        nc.vector.bn_stats(out=st6[:, hf, :],
                                           in_=ht[:, hf * 512:(hf + 1) * 512])
                    mv = p1s.tile([128, 2], F32, tag="mv")
                    nc.vector.bn_aggr(out=mv, in_=st6)
                    vpe = p1s.tile([128, 1], F32, tag="vpe")
                    nc.gpsimd.tensor_scalar(out=vpe, in0=mv[:, 1:2],
                                            scalar1=LN_EPS, scalar2=None,
                                            op0=AX.add)
                    rstd = p1s.tile([128, 1], F32, tag="rstd")
                    nc.gpsimd.tensor_tensor(out=rstd, in0=vpe, in1=nhalf_s,
                                            op=AX.pow)
                    hn = p1n.tile([128, H], BF16, tag="hn")
                    nc.vector.tensor_scalar(out=hn, in0=ht, scalar1=mv[:, 0:1],
                                            scalar2=rstd, op0=AX.subtract,
                                            op1=AX.mult)

                    hTb = p1t.tile([128, 8, 128], BF16, tag="hTb")
                    nc.sync.dma_start_transpose(hTb, hn)

                    qkt = qkvp.tile([128, 2, 512], F32, tag="qk")
                    vt = vkvp.tile([128, 512], F32, tag="v")
                    # p-state keep-warm fillers: depend only on consts, land in
                    # the region the real bias matmul resets with start=True
                    for _ in range(FILLERS):
                        nc.tensor.matmul(qkt[:, 0, 0:128], lhsT=onesr_s,
                                         rhs=wb_s[:, 0:128], start=True,
                                         stop=True)
                    for o in range(3):
                        dst = vt if o == 2 else qkt[:, o, :]
                        nc.tensor.matmul(dst, lhsT=onesr_s,
                                         rhs=wb_s[:, o * 512:(o + 1) * 512],
                                         start=True, stop=False)
                        for c in range(8):
                            nc.tensor.matmul(
                                dst, lhsT=hTb[:, c, :],
                                rhs=w_s[:, c, o * 512:(o + 1) * 512],
                                start=False, stop=(c == 7))

                    pending_c.append((t, qkt, vt))
                    if len(pending_c) > 1:
                        tc_, qkt_, vt_ = pending_c.pop(0)
                        pending.append((tc_, copy_block(tc_, qkt_, vt_)))
                    if len(pending) > 1:
                        rope_block(*pending.pop(0))
                for tc_, qkt_, vt_ in pending_c:
                    pending.append((tc_, copy_block(tc_, qkt_, vt_)))
                for item in pending:
                    rope_block(*item)

            # ---------------- Phase 2 ----------------
            if "2" not in phases:
                nc.compile()
                return nc
            with tc.tile_pool(name="p2x", bufs=2) as p2x, \
                 tc.tile_pool(name="p2o", bufs=4) as p2o, \
                 tc.tile_pool(name="spp", bufs=3, space="PSUM") as spp, \
                 tc.tile_pool(name="ctxp", bufs=2, space="PSUM") as ctxp:

                units = [(h, qh) for h in range(HPC) for qh in range(2)]

                def emit_scores(ui):
                    h, qh = units[ui]
                    pb = 64 * (h % 2)
                    blk = h // 2
                    Ut = U[ui % 2]
                    for kc in range(TOKCH):
                        sp = spp.tile([128, 1024], F32, tag="sp")
                        for nn in range(2):
                            nc.tensor.matmul(
                                sp[:, nn * 512:(nn + 1) * 512],
                                lhsT=kT[pb:pb + 64, blk, kc, :],
                                rhs=qT[pb:pb + 64, blk,
                                       qh * 8 + nn * 4:qh * 8 + (nn + 1) * 4, :],
                                start=True, stop=True,
                                tile_position=(pb, 0))
                        if kc in CHAIN_KCS:
                            xsb = p2x.tile([128, 1024], F32, tag="xsb")
                            nc.vector.tensor_copy(xsb, sp)
                            two_ap = _ap(two_s, 0, [(0, 512), (1, 2)])
                            nc.gpsimd.tensor_tensor(out=Ut[:, kc, :],
                                                    in0=two_ap, in1=xsb,
                                                    op=AX.pow)
                        else:
                            nc.scalar.activation(Ut[:, kc, :], sp, ACT.Exp,
                                                 scale=LN2)

                def emit_ctx(ui):
                    h, qh = units[ui]
                    Ut = U[ui % 2]
                    cps = []
                    for nn in range(2):
                        cpd = ctxp.tile([65, 512], F32, tag="cpd")
                        for kc in range(TOKCH):
                            nc.tensor.matmul(
                                cpd, lhsT=vA[:, kc, h, :],
                                rhs=Ut[:, kc, nn * 512:(nn + 1) * 512],
                                start=(kc == 0), stop=(kc == TOKCH - 1))
                        cps.append(cpd)
                    for nn in range(2):
                        ctxs = p2o.tile([65, 512], BF16, tag="ctxs")
                        nc.vector.tensor_copy(ctxs, cps[nn])
                        nc.sync.dma_start(out=out_d[ui, nn, :, :], in_=ctxs)

                emit_scores(0)
                for ui in range(1, len(units)):
                    emit_scores(ui)
                    emit_ctx(ui - 1)
                emit_ctx(len(units) - 1)

    nc.compile()
    return nc


def _host_inputs(hidden_states, sinusoidal_pos, ln_weight, ln_bias, w_qkv, b_qkv):
    import ml_dtypes
    bf16 = ml_dtypes.bfloat16

    hidden_states = np.ascontiguousarray(hidden_states, dtype=np.float32)
    w_qkv = np.asarray(w_qkv, dtype=np.float32)
    b_qkv = np.asarray(b_qkv, dtype=np.float32)
    ln_weight = np.asarray(ln_weight, dtype=np.float32)
    ln_bias = np.asarray(ln_bias, dtype=np.float32)
    sp = np.asarray(sinusoidal_pos, dtype=np.float32).reshape(S, HD)

    w_eff = ln_weight[:, None] * w_qkv
    b_eff = b_qkv + ln_bias @ w_qkv

    sin_tab = sp[:, :32]
    cos_tab = sp[:, 32:]

    # tables, token-major: [128 tok-in-chunk, chunk, ...]
    cosR = cos_tab.reshape(TOKCH, 128, 32).transpose(1, 0, 2)   # [128, tc, 32]
    sinR = sin_tab.reshape(TOKCH, 128, 32).transpose(1, 0, 2)
    cq = cosR * CEXP
    ck = cosR.copy()
    sq = np.stack([-sinR, sinR], axis=2) * CEXP                 # [128, tc, 2, 32]
    sk = np.stack([-sinR, sinR], axis=2)

    # col order per head: [even d | odd d]
    hh, ff, ee = np.meshgrid(np.arange(8), np.arange(2), np.arange(32),
                             indexing="ij")
    localperm = (hh * 64 + 2 * ee + ff).reshape(-1)             # [512]

    in_maps = []
    for core in range(N_CORES):
        b = core // 2
        g = core % 2
        heads = g * 8 + np.arange(8)
        qcols = heads[localperm // 64] * 64 + localperm % 64
        kcols = 1024 + qcols
        vcols = 2048 + np.repeat(heads, 64) * 64 + np.tile(np.arange(64), 8)
        colsel = np.concatenate([qcols, kcols, vcols])

        wsel = w_eff[:, colsel]                                 # [1024, 1536]
        w3 = wsel.reshape(8, 128, 1536).transpose(1, 0, 2)      # [128, 8, 1536]

        in_maps.append({
            "hid": hidden_states[b],
            "w": np.ascontiguousarray(w3).astype(bf16),
            "wb": b_eff[colsel][None, :].astype(bf16),
            "onesr": np.ones((1, 128), np.float32).astype(bf16),
            "cq": np.ascontiguousarray(cq, np.float32).astype(bf16),
            "sq": np.ascontiguousarray(sq, np.float32).astype(bf16),
            "ck": np.ascontiguousarray(ck, np.float32).astype(bf16),
            "sk": np.ascontiguousarray(sk, np.float32).astype(bf16),
            "two": np.full((128, 2), 2.0, np.float32),
        })
    return in_maps


def _run(trace=False, **inputs):
    if "nc" not in _CACHE:
        _CACHE["nc"] = _build_program()
    nc = _CACHE["nc"]
    in_maps = _host_inputs(**inputs)
    res = run_bass_kernel_spmd(nc, in_maps, core_ids=list(range(N_CORES)),
                               trace=trace)
    out = np.empty((B, S, H), np.float32)
    for core in range(N_CORES):
        b = core // 2
        g = core % 2
        o = np.asarray(res.results[core]["out"], dtype=np.float32)
        for h in range(HPC):
            Hh = g * 8 + h
            for qh in range(2):
                ui = h * 2 + qh
                for nn in range(2):
                    num = o[ui, nn, 0:64, :]
                    den = o[ui, nn, 64, :]
                    toks = qh * 1024 + nn * 512
                    out[b, toks:toks + 512, Hh * 64:(Hh + 1) * 64] = \
                        (num / den[None, :]).T
    return out, res


def kernel(**inputs):
    out, _ = _run(trace=False, **inputs)
    return out



# revision 25
# speedup vs baseline: 1.0788x; 1.0788x over previous
"""RoFormer self-attention v3: all-bf16 matmuls + DMA transposes +
multi-engine softmax exp, on 8 trn2 cores.

Sharding: core c -> batch b = c//2, head-group g = c%2 (8 of 16 heads).

Per core:
  Phase 1 (per 128-token chunk t, 16 chunks):
    LN stats (DVE bn_stats) -> hn = (ht-mu)*rstd bf16 (DVE tensor_scalar, 2x)
    dmaT hn -> hTb [128, 8, 128] bf16 (lhsT for QKV directly)
    QKV: per output o: ones-row bias matmul + 8 bf16 matmuls [128,128]x[128,512]
    RoPE on PSUM q/k (DVE): col order per head = [even d | odd d] so the
      rotate-half partner is a contiguous 32-col block; q tables carry
      C = log2(e)/8 so scores come out in exp2 units
    dmaT qf/kf -> qT/kT [128, 4, 16, 128] bf16 (partition 64*(h%2)+d')
    v: PSUM -> vA [128, 16, 8, 65] bf16 + ones column
  Phase 2 (per unit u = (head h, query-half qh), 16 units):
    scores: 2 bf16 matmuls lhsT=kT [64,128] rhs=qT [64,512] -> sp [128,1024]
    exp: 2^sp -> U[u%2][:, kc, :] bf16; split ACT Exp(scale=ln2) vs
         chain (DVE copy -> Pool pow(2,x))
    ctx: 16 accumulating matmuls lhsT=vA [128, 65] -> cpd [65, 512] x2
    tail: copy cpd -> bf16 SBUF, DMA out.  Softmax division happens on HOST.
"""

import numpy as np

import concourse.bass as bass
import concourse.mybir as mybir
import concourse.tile as tile
from concourse import bacc
from concourse.bass_utils import run_bass_kernel_spmd

F32 = mybir.dt.float32
BF16 = mybir.dt.bfloat16
AX = mybir.AluOpType
ACT = mybir.ActivationFunctionType

B, S, H = 4, 2048, 1024
NH, HD = 16, 64
LN_EPS = 1e-12
N_CORES = 8
HPC = 8
TOKCH = 16
CEXP = float(np.log2(np.e) / np.sqrt(HD))
LN2 = float(np.log(2.0))

# chain (DVE-copy + Pool-pow) exp blocks: stride-3 spread keeps the Pool
# load even across the unit AND is expressible as one regular AP for the
# per-unit U fence.
CHAIN_KCS = (0, 3, 6, 9, 12, 15)
import os
FILLERS = int(os.environ.get('FILLERS', '0'))
WARMUPS = int(os.environ.get('WARMUPS', '12'))

_CACHE = {}


def _ap(t, offset_elems, free_dims):
    return bass.AP(tensor=t.tensor, offset=t.offset + offset_elems,
                   ap=[[t.ap[0][0], t.ap[0][1]]] + [[s, n] for s, n in free_dims])


def _build_program(phases="12"):
    nc = bacc.Bacc("TRN2", target_bir_lowering=False)

    hid_d = nc.dram_tensor("hid", [S, H], BF16, kind="ExternalInput")
    w_d = nc.dram_tensor("w", [128, 8, 1536], BF16, kind="ExternalInput")
    wb_d = nc.dram_tensor("wb", [1, 1536], BF16, kind="ExternalInput")
    onesr_d = nc.dram_tensor("onesr", [1, 128], BF16, kind="ExternalInput")
    cq_d = nc.dram_tensor("cq", [128, TOKCH, 32], BF16, kind="ExternalInput")
    sq_d = nc.dram_tensor("sq", [128, TOKCH, 2, 32], BF16, kind="ExternalInput")
    ck_d = nc.dram_tensor("ck", [128, TOKCH, 32], BF16, kind="ExternalInput")
    sk_d = nc.dram_tensor("sk", [128, TOKCH, 2, 32], BF16, kind="ExternalInput")
    two_d = nc.dram_tensor("two", [128, 2], F32, kind="ExternalInput")
    out_d = nc.dram_tensor("out", [HPC, 128, 2, 8, 65], BF16,
                           kind="ExternalOutput")

    with tile.TileContext(nc) as tc:
        with tc.tile_pool(name="const", bufs=1) as const, \
             tc.tile_pool(name="store", bufs=1) as store:
            w_s = const.tile([128, 8, 1536], BF16)
            wb_s = const.tile([1, 1536], BF16)
            onesr_s = const.tile([1, 128], BF16)
            cq_s = const.tile([128, TOKCH, 32], BF16)
            sq_s = const.tile([128, TOKCH, 2, 32], BF16)
            ck_s = const.tile([128, TOKCH, 32], BF16)
            sk_s = const.tile([128, TOKCH, 2, 32], BF16)
            two_s = const.tile([128, 2], F32)
            nhalf_s = const.tile([128, 1], F32)
            onesc_s = const.tile([128, 1], BF16)

            qT = store.tile([128, 4, TOKCH, 128], BF16)
            kT = store.tile([128, 4, TOKCH, 128], BF16)
            vA = store.tile([128, TOKCH, HPC, 65], BF16)
            U0 = store.tile([128, TOKCH, 1024], BF16)
            U1 = store.tile([128, TOKCH, 1024], BF16)
            U = [U0, U1]

            # ---------------- Phase 1 ----------------
            with tc.tile_pool(name="p1h", bufs=4) as p1h, \
                 tc.tile_pool(name="p1s", bufs=3) as p1s, \
                 tc.tile_pool(name="p1n", bufs=2) as p1n, \
                 tc.tile_pool(name="p1t", bufs=3) as p1t, \
                 tc.tile_pool(name="p1r", bufs=4) as p1r, \
                 tc.tile_pool(name="qkvp", bufs=3, space="PSUM") as qkvp, \
                 tc.tile_pool(name="vkvp", bufs=2, space="PSUM") as vkvp:
                # consts on the gpsimd SWDGE queue, w (chunked) on SP, ht
                # loads on ACT: three queues so the critical chunk-0
                # transpose isn't stuck behind bulk loads.
                nc.gpsimd.dma_start(out=wb_s, in_=wb_d[:, :])
                nc.gpsimd.dma_start(out=onesr_s, in_=onesr_d[:, :])
                nc.gpsimd.memset(nhalf_s, -0.5)
                nc.gpsimd.memset(onesc_s, 1.0)
                wu = qkvp.tile([128, 2, 512], F32, tag="qk")
                for _ in range(WARMUPS):
                    nc.tensor.matmul(wu[:, 0, :], lhsT=onesr_s,
                                     rhs=wb_s[:, 0:512], start=True, stop=True)
                for c in range(8):
                    nc.sync.dma_start(out=w_s[:, c, :], in_=w_d[:, c, :])
                nc.gpsimd.dma_start(out=cq_s, in_=cq_d[:, :, :])
                nc.gpsimd.dma_start(out=sq_s, in_=sq_d[:, :, :, :])
                nc.gpsimd.dma_start(out=ck_s, in_=ck_d[:, :, :])
                nc.gpsimd.dma_start(out=sk_s, in_=sk_d[:, :, :, :])
                nc.gpsimd.dma_start(out=two_s, in_=two_d[:, :])

                def copy_block(t, qkt, vt):
                    # q/k PSUM -> SBUF bf16 (ACT) frees PSUM fast; v -> vA
                    qkb = p1r.tile([128, 2, 512], BF16, tag="qkb")
                    nc.scalar.copy(qkb, qkt)
                    nc.scalar.copy(
                        vA[:, t, :, 0:64],
                        vt.rearrange("p (h d) -> p h d", d=64))
                    nc.gpsimd.memset(vA[:, t, :, 64:65], 1.0)
                    return qkb

                def rope_block(t, qkb):
                    for qk in range(2):
                        cos_s = cq_s if qk == 0 else ck_s
                        sin_s = sq_s if qk == 0 else sk_s
                        src = qkb[:, qk, :]
                        qc = p1r.tile([128, 8, 2, 32], BF16, tag=f"qc{qk}")
                        cos_ap = _ap(cos_s, t * 32, [(0, 8), (0, 2), (1, 32)])
                        nc.vector.tensor_tensor(
                            out=qc,
                            in0=src.rearrange("p (h f e) -> p h f e", f=2, e=32),
                            in1=cos_ap, op=AX.mult)
                        sw_ap = _ap(qkb, qk * 512 + 32,
                                    [(64, 8), (-32, 2), (1, 32)])
                        sin_ap = _ap(sin_s, t * 64, [(0, 8), (32, 2), (1, 32)])
                        t2 = p1r.tile([128, 8, 2, 32], BF16, tag=f"t2{qk}")
                        nc.vector.tensor_tensor(out=t2, in0=sw_ap, in1=sin_ap,
                                                op=AX.mult)
                        qf = p1r.tile([128, 512], BF16, tag=f"qf{qk}")
                        nc.vector.tensor_tensor(
                            out=qf.rearrange("p (h f e) -> p h f e", f=2, e=32),
                            in0=qc, in1=t2, op=AX.add)
                        dst = qT if qk == 0 else kT
                        nc.sync.dma_start_transpose(dst[:, :, t, :], qf)

                pending = []
                pending_c = []
                ht_tiles = {}
                for tpre in range(4):
                    ht = p1h.tile([128, H], BF16, tag="ht")
                    nc.scalar.dma_start(out=ht,
                                        in_=hid_d[tpre * 128:(tpre + 1) * 128, :])
                    ht_tiles[tpre] = ht

                for t in range(TOKCH):
                    if t + 4 < TOKCH:
                        htn = p1h.tile([128, H], BF16, tag="ht")
                        nc.scalar.dma_start(
                            out=htn, in_=hid_d[(t + 4) * 128:(t + 5) * 128, :])
                        ht_tiles[t + 4] = htn
                    ht = ht_tiles.pop(t)

                    st6 = p1s.tile([128, 2, 6], F32, tag="st6")
                    for hf in range(2):
                        nc.vector.bn_stats(out=st6[:, hf, :],
                                           in_=ht[:, hf * 512:(hf + 1) * 512])
                    mv = p1s.tile([128, 2], F32, tag="mv")
                    nc.vector.bn_aggr(out=mv, in_=st6)
                    vpe = p1s.tile([128, 1], F32, tag="vpe")
                    nc.gpsimd.tensor_scalar(out=vpe, in0=mv[:, 1:2],
                                            scalar1=LN_EPS, scalar2=None,
                                            op0=AX.add)
                    rstd = p1s.tile([128, 1], F32, tag="rstd")
                    nc.gpsimd.tensor_tensor(out=rstd, in0=vpe, in1=nhalf_s,
                                            op=AX.pow)
                    hn = p1n.tile([128, H], BF16, tag="hn")
                    nc.vector.tensor_scalar(out=hn, in0=ht, scalar1=mv[:, 0:1],
                                            scalar2=rstd, op0=AX.subtract,
                                            op1=AX.mult)

                    hTb = p1t.tile([128, 8, 128], BF16, tag="hTb")
                    nc.sync.dma_start_transpose(hTb, hn)

                    qkt = qkvp.tile([128, 2, 512], F32, tag="qk")
                    vt = vkvp.tile([128, 512], F32, tag="v")
                    # hTb fence: read the fresh transpose as MOVING operand so
                    # the in-order PE can't run the QKV ldweights before the
                    # DMA transpose lands (hw ldweights ignore matmult waits).
                    # Output lands in a region the o=0 bias matmul (start=True)
                    # overwrites right after.
                    nc.tensor.matmul(qkt[0:1, 0, 0:8], lhsT=onesc_s,
                                     rhs=hTb[:, :, 0:1], start=True, stop=True)
                    # p-state keep-warm fillers: depend only on consts, land in
                    # the region the real bias matmul resets with start=True
                    for _ in range(FILLERS):
                        nc.tensor.matmul(qkt[:, 0, 0:128], lhsT=onesr_s,
                                         rhs=wb_s[:, 0:128], start=True,
                                         stop=True)
                    for o in range(3):
                        # v bias is added on the host after the softmax
                        # division (it rides through as b*den/den), so only
                        # q/k get the ones-row bias matmul.
                        dst = vt if o == 2 else qkt[:, o, :]
                        if o < 2:
                            nc.tensor.matmul(dst, lhsT=onesr_s,
                                             rhs=wb_s[:, o * 512:(o + 1) * 512],
                                             start=True, stop=False)
                        for c in range(8):
                            nc.tensor.matmul(
                                dst, lhsT=hTb[:, c, :],
                                rhs=w_s[:, c, o * 512:(o + 1) * 512],
                                start=(o == 2 and c == 0), stop=(c == 7))

                    pending_c.append((t, qkt, vt))
                    if len(pending_c) > 1:
                        tc_, qkt_, vt_ = pending_c.pop(0)
                        pending.append((tc_, copy_block(tc_, qkt_, vt_)))
                    if len(pending) > 1:
                        rope_block(*pending.pop(0))
                for tc_, qkt_, vt_ in pending_c:
                    pending.append((tc_, copy_block(tc_, qkt_, vt_)))
                for item in pending:
                    rope_block(*item)

            # ---------------- Phase 2 ----------------
            if "2" not in phases:
                nc.compile()
                return nc
            with tc.tile_pool(name="p2st", bufs=1) as p2st, \
                 tc.tile_pool(name="p2x", bufs=2) as p2x, \
                 tc.tile_pool(name="spp", bufs=3, space="PSUM") as spp, \
                 tc.tile_pool(name="ctxp", bufs=2, space="PSUM") as ctxp:

                outst = p2st.tile([128, HPC, 2, 8, 65], BF16)
                units = [(h, qh) for h in range(HPC) for qh in range(2)]

                def scores_block(ui, kc):
                    h, qh = units[ui]
                    pb = 64 * (h % 2)
                    blk = h // 2
                    Ut = U[ui % 2]
                    sp = spp.tile([128, 1024], F32, tag="sp")
                    for nn in range(2):
                        nc.tensor.matmul(
                            sp[:, nn * 512:(nn + 1) * 512],
                            lhsT=kT[pb:pb + 64, blk, kc, :],
                            rhs=qT[pb:pb + 64, blk,
                                   qh * 8 + nn * 4:qh * 8 + (nn + 1) * 4, :],
                            start=True, stop=True,
                            tile_position=(pb, 0))
                    if kc in CHAIN_KCS:
                        xsb = p2x.tile([128, 1024], F32, tag="xsb")
                        nc.vector.tensor_copy(xsb, sp)
                        two_ap = _ap(two_s, 0, [(0, 512), (1, 2)])
                        nc.gpsimd.tensor_tensor(out=Ut[:, kc, :],
                                                in0=two_ap, in1=xsb,
                                                op=AX.pow)
                    else:
                        nc.scalar.activation(Ut[:, kc, :], sp, ACT.Exp,
                                             scale=LN2)

                def ctx_qchunk(ui, qc):
                    # transposed ctx: out partitions = 128 query tokens,
                    # free = 64 head dims + 1 denominator column.  The tile is
                    # padded to a full 2KB PSUM bank so concurrent accumulation
                    # groups never share a bank.
                    h, qh = units[ui]
                    Ut = U[ui % 2]
                    cpd = ctxp.tile([128, 512], F32, tag="cpd")
                    if qc == 0:
                        # PE fences: on real hardware the weights load
                        # (ldweights) can execute ahead of the matmult's
                        # semaphore wait, so before the first U ldweights of
                        # this unit, read every U kc-slice as the MOVING
                        # operand of throwaway matmuls.  One fence per
                        # producer engine class (Pool chain kcs first, then
                        # all) so each needs a single semaphore wait.
                        chain_ap = _ap(Ut, 0, [(1024 * 3, 6), (1, 1)])
                        nc.tensor.matmul(cpd[0:1, 480:486],
                                         lhsT=onesc_s, rhs=chain_ap,
                                         start=True, stop=True)
                        nc.tensor.matmul(cpd[0:1, 496:512],
                                         lhsT=onesc_s,
                                         rhs=Ut[:, :, 0:1],
                                         start=True, stop=True)
                    for kc in range(TOKCH):
                        nc.tensor.matmul(
                            cpd[:, 0:65], lhsT=Ut[:, kc, qc * 128:(qc + 1) * 128],
                            rhs=vA[:, kc, h, :],
                            start=(kc == 0), stop=(kc == TOKCH - 1))
                    nc.vector.tensor_copy(outst[:, h, qh, qc, :], cpd[:, 0:65])

                def head_dma(h):
                    nc.sync.dma_start(out=out_d[h], in_=outst[:, h])

                # ctx of unit ui-1 is interleaved into unit ui's scores from
                # kc=3 on: the sp-pool WAR (bufs=3) forces exp(ui-1,15) to
                # complete before scores(ui,2) executes, so U[ui-1] is fully
                # written before any of its ldweights stream in.
                CTX_SLOTS = {3: 0, 4: 1, 6: 2, 8: 3, 10: 4, 12: 5, 14: 6, 15: 7}
                # phase-2 entry fence: kT is only ever a stationary operand,
                # so force the PE behind the last kf/qf DMA transposes by
                # reading one column of every chunk as a moving operand.
                spf = spp.tile([128, 1024], F32, tag="sp")
                nc.tensor.matmul(spf[0:1, 0:64], lhsT=onesc_s,
                                 rhs=kT[:, :, :, 0:1], start=True, stop=True)
                nc.tensor.matmul(spf[0:1, 64:128], lhsT=onesc_s,
                                 rhs=qT[:, :, :, 0:1], start=True, stop=True)

                nu = len(units)
                for ui in range(nu):
                    for kc in range(TOKCH):
                        scores_block(ui, kc)
                        if ui > 0 and kc in CTX_SLOTS:
                            ctx_qchunk(ui - 1, CTX_SLOTS[kc])
                    if ui > 0 and units[ui - 1][1] == 1:
                        head_dma(units[ui - 1][0])
                for qc in range(8):
                    ctx_qchunk(nu - 1, qc)
                head_dma(HPC - 1)

    nc.compile()
    return nc


def _host_inputs(hidden_states, sinusoidal_pos, ln_weight, ln_bias, w_qkv, b_qkv):
    import ml_dtypes
    bf16 = ml_dtypes.bfloat16

    hidden_states = np.ascontiguousarray(hidden_states, dtype=np.float32)
    w_qkv = np.asarray(w_qkv, dtype=np.float32)
    b_qkv = np.asarray(b_qkv, dtype=np.float32)
    ln_weight = np.asarray(ln_weight, dtype=np.float32)
    ln_bias = np.asarray(ln_bias, dtype=np.float32)
    sp = np.asarray(sinusoidal_pos, dtype=np.float32).reshape(S, HD)

    w_eff = ln_weight[:, None] * w_qkv
    b_eff = b_qkv + ln_bias @ w_qkv

    sin_tab = sp[:, :32]
    cos_tab = sp[:, 32:]

    # tables, token-major: [128 tok-in-chunk, chunk, ...]
    cosR = cos_tab.reshape(TOKCH, 128, 32).transpose(1, 0, 2)   # [128, tc, 32]
    sinR = sin_tab.reshape(TOKCH, 128, 32).transpose(1, 0, 2)
    cq = cosR * CEXP
    ck = cosR.copy()
    sq = np.stack([-sinR, sinR], axis=2) * CEXP                 # [128, tc, 2, 32]
    sk = np.stack([-sinR, sinR], axis=2)

    # col order per head: [even d | odd d]
    hh, ff, ee = np.meshgrid(np.arange(8), np.arange(2), np.arange(32),
                             indexing="ij")
    localperm = (hh * 64 + 2 * ee + ff).reshape(-1)             # [512]

    in_maps = []
    for core in range(N_CORES):
        b = core // 2
        g = core % 2
        heads = g * 8 + np.arange(8)
        qcols = heads[localperm // 64] * 64 + localperm % 64
        kcols = 1024 + qcols
        vcols = 2048 + np.repeat(heads, 64) * 64 + np.tile(np.arange(64), 8)
        colsel = np.concatenate([qcols, kcols, vcols])

        wsel = w_eff[:, colsel]                                 # [1024, 1536]
        w3 = wsel.reshape(8, 128, 1536).transpose(1, 0, 2)      # [128, 8, 1536]

        in_maps.append({
            "hid": hidden_states[b].astype(bf16),
            "w": np.ascontiguousarray(w3).astype(bf16),
            "wb": b_eff[colsel][None, :].astype(bf16),
            "onesr": np.ones((1, 128), np.float32).astype(bf16),
            "cq": np.ascontiguousarray(cq, np.float32).astype(bf16),
            "sq": np.ascontiguousarray(sq, np.float32).astype(bf16),
            "ck": np.ascontiguousarray(ck, np.float32).astype(bf16),
            "sk": np.ascontiguousarray(sk, np.float32).astype(bf16),
            "two": np.full((128, 2), 2.0, np.float32),
        })
    return in_maps


def _run(trace=False, **inputs):
    if "nc" not in _CACHE:
        _CACHE["nc"] = _build_program()
    nc = _CACHE["nc"]
    in_maps = _host_inputs(**inputs)
    res = run_bass_kernel_spmd(nc, in_maps, core_ids=list(range(N_CORES)),
                               trace=trace)

    # v bias is applied here: ctx = sum_k p_k (v_k + b) = num/den + b
    ln_bias = np.asarray(inputs["ln_bias"], dtype=np.float32)
    w_qkv = np.asarray(inputs["w_qkv"], dtype=np.float32)
    b_qkv = np.asarray(inputs["b_qkv"], dtype=np.float32)
    b_eff = b_qkv + ln_bias @ w_qkv
    bias_v = b_eff[2 * H:3 * H]                          # [1024], head-major

    out = np.empty((B, S, H), np.float32)
    for core in range(N_CORES):
        b = core // 2
        g = core % 2
        o = np.asarray(res.results[core]["out"], dtype=np.float32)
        o = o.reshape(HPC, 128, 2, 8, 65)
        blk = o[..., :64] / o[..., 64:65]                # [h, row, qh, qc, d]
        blk = blk.transpose(2, 3, 1, 0, 4).reshape(S, HPC * 64)
        out[b, :, g * 512:(g + 1) * 512] = \
            blk + bias_v[None, g * 512:(g + 1) * 512]
    return out, res


def kernel(**inputs):
    out, _ = _run(trace=False, **inputs)
    return out



# revision 77
# speedup vs baseline: 1.1579x; 1.0733x over previous
"""RoFormer self-attention v4: fp8 DoubleRow QKV + PE transposes +
transposed ctx + multi-engine softmax exp, on 8 trn2 cores.

Sharding: core c -> batch b = c//2, head-group g = c%2 (8 of 16 heads).

Per core:
  Phase 1 (per 128-token chunk t, 16 chunks):
    LN stats (DVE bn_stats) -> hn = (ht-mu)*rstd bf16; ht arrives bf16
    hn -> hT via 8 PE identity-transpose matmuls (bf16 PSUM) -> fp8 hi
      (ACT copy) + fp8 lo = hn - hi (DVE): residual split keeps accuracy
    QKV = hi@w_hi + hi@w_lo + lo@w_hi with fp8 DoubleRow matmuls (2
      k-chunks per instr, 0.5 cyc/col); w_hi/w_lo are host-split fp8 of
      32x-scaled weights, ACT copies scale PSUM by 1/32 on the way out
    RoPE on q/k (DVE): col order per head = [even d | odd d] so the
      rotate-half partner is a contiguous 32-col block; q tables carry
      C = log2(e)/8 so scores come out in exp2 units
    dmaT qf/kf -> qT/kT [128, 4, 16, 128] bf16 (partition 64*(h%2)+d')
    v: PSUM -> vA [128, 16, 8, 65] bf16 + ones column (no bias: v bias is
      added on the host after the softmax division)
  Phase 2 (per unit u = (head h, query-half qh), 16 units):
    scores: 2 bf16 matmuls lhsT=kT [64,128] rhs=qT [64,512] -> sp [128,1024]
    exp: 2^sp -> U[u%2][:, kc, :] bf16; split ACT Exp(scale=ln2) vs
         chain (DVE copy -> Pool pow(2,x)) on CHAIN_KCS
    ctx (transposed): per 128-query chunk, 16 accumulating matmuls
      lhsT=U[keys,q] rhs=vA[keys,65] -> cpd [128 q, 65]; full partition
      use halves the PE cost vs the [65, 512] orientation
    tail: DVE copy -> out staging, one DMA per head.  Division on HOST.

Hardware-race note: on real trn2 the PE weight load (ldweights) does NOT
honor the matmult's semaphore waits, so every freshly-written stationary
operand (hn, hi, lo, U) is preceded by a tiny "fence" matmul that reads
it as the MOVING operand (one fence per producer engine).
"""

import numpy as np

import concourse.bass as bass
import concourse.mybir as mybir
import concourse.tile as tile
from concourse import bacc
from concourse.bass_utils import run_bass_kernel_spmd
from concourse.masks import make_identity

F32 = mybir.dt.float32
BF16 = mybir.dt.bfloat16
FP8 = mybir.dt.float8e4
WSCALE = 32.0
AX = mybir.AluOpType
ACT = mybir.ActivationFunctionType

B, S, H = 4, 2048, 1024
NH, HD = 16, 64
LN_EPS = 1e-12
N_CORES = 8
HPC = 8
TOKCH = 16
CEXP = float(np.log2(np.e) / np.sqrt(HD))
LN2 = float(np.log(2.0))

# chain (DVE-copy + Pool-pow) exp blocks: stride-2 spread over the EARLY
# kcs only -- the chain path has ~2.7us latency after its scores block, so
# the tail kcs must use the fast ACT path or the next unit's U fence
# stalls the PE.  Regular stride keeps the fence AP expressible.
CHAIN_KCS = (0, 2, 4, 6, 8, 10)
import os
FILLERS = int(os.environ.get('FILLERS', '0'))
WARMUPS = int(os.environ.get('WARMUPS', '12'))

_CACHE = {}


def _ap(t, offset_elems, free_dims):
    return bass.AP(tensor=t.tensor, offset=t.offset + offset_elems,
                   ap=[[t.ap[0][0], t.ap[0][1]]] + [[s, n] for s, n in free_dims])


def _build_program(phases="12"):
    nc = bacc.Bacc("TRN2", target_bir_lowering=False)

    hid_d = nc.dram_tensor("hid", [S, H], BF16, kind="ExternalInput")
    whi_d = nc.dram_tensor("whi", [128, 8, 1536], FP8, kind="ExternalInput")
    wlo_d = nc.dram_tensor("wlo", [128, 8, 1536], FP8, kind="ExternalInput")
    wb_d = nc.dram_tensor("wb", [1, 1536], BF16, kind="ExternalInput")
    onesr_d = nc.dram_tensor("onesr", [1, 128], BF16, kind="ExternalInput")
    cq_d = nc.dram_tensor("cq", [128, TOKCH, 32], BF16, kind="ExternalInput")
    sq_d = nc.dram_tensor("sq", [128, TOKCH, 2, 32], BF16, kind="ExternalInput")
    ck_d = nc.dram_tensor("ck", [128, TOKCH, 32], BF16, kind="ExternalInput")
    sk_d = nc.dram_tensor("sk", [128, TOKCH, 2, 32], BF16, kind="ExternalInput")
    two_d = nc.dram_tensor("two", [128, 2], F32, kind="ExternalInput")
    out_d = nc.dram_tensor("out", [HPC, 128, 2, 8, 65], BF16,
                           kind="ExternalOutput")

    with tile.TileContext(nc) as tc:
        with tc.tile_pool(name="const", bufs=1) as const, \
             tc.tile_pool(name="store", bufs=1) as store:
            whi_s = const.tile([128, 8, 1536], FP8)
            wlo_s = const.tile([128, 8, 1536], FP8)
            wb_s = const.tile([1, 1536], BF16)
            onesr_s = const.tile([1, 128], BF16)
            cq_s = const.tile([128, TOKCH, 32], BF16)
            sq_s = const.tile([128, TOKCH, 2, 32], BF16)
            ck_s = const.tile([128, TOKCH, 32], BF16)
            sk_s = const.tile([128, TOKCH, 2, 32], BF16)
            two_s = const.tile([128, 2], F32)
            nhalf_s = const.tile([128, 1], F32)
            onesc_s = const.tile([128, 1], BF16)
            ident_s = const.tile([128, 128], BF16)

            qT = store.tile([128, 4, TOKCH, 128], BF16)
            kT = store.tile([128, 4, TOKCH, 128], BF16)
            vA = store.tile([128, TOKCH, HPC, 65], BF16)
            U0 = store.tile([128, TOKCH, 1024], BF16)
            U1 = store.tile([128, TOKCH, 1024], BF16)
            U = [U0, U1]

            # ---------------- Phase 1 ----------------
            with tc.tile_pool(name="p1h", bufs=4) as p1h, \
                 tc.tile_pool(name="p1s", bufs=4) as p1s, \
                 tc.tile_pool(name="p1n", bufs=2) as p1n, \
                 tc.tile_pool(name="p1t", bufs=4) as p1t, \
                 tc.tile_pool(name="p1r", bufs=4) as p1r, \
                 tc.tile_pool(name="qkvp", bufs=2, space="PSUM") as qkvp, \
                 tc.tile_pool(name="vkvp", bufs=2, space="PSUM") as vkvp, \
                 tc.tile_pool(name="tpp", bufs=2, space="PSUM") as tpp:
                # consts on the gpsimd SWDGE queue, w (chunked) on SP, ht
                # loads on ACT: three queues so the critical chunk-0
                # transpose isn't stuck behind bulk loads.
                nc.sync.dma_start(out=wb_s, in_=wb_d[:, :])
                nc.sync.dma_start(out=onesr_s, in_=onesr_d[:, :])
                nc.gpsimd.memset(nhalf_s, -0.5)
                nc.gpsimd.memset(onesc_s, 1.0)
                make_identity(nc, ident_s)
                wu = qkvp.tile([128, 2, 512], F32, tag="qk")
                for _ in range(WARMUPS):
                    nc.tensor.matmul(wu[:, 0, :], lhsT=onesr_s,
                                     rhs=wb_s[:, 0:512], start=True, stop=True)
                # only the first two w chunks load ahead of the chunk-0
                # transpose; the rest are issued after it (see chunk loop) so
                # the serialized DMA device reaches hTb(0) early.
                for c in range(2):
                    nc.sync.dma_start(out=whi_s[:, c, :], in_=whi_d[:, c, :])
                    nc.sync.dma_start(out=wlo_s[:, c, :], in_=wlo_d[:, c, :])
                nc.gpsimd.dma_start(out=cq_s, in_=cq_d[:, :, :])
                nc.gpsimd.dma_start(out=sq_s, in_=sq_d[:, :, :, :])
                nc.gpsimd.dma_start(out=ck_s, in_=ck_d[:, :, :])
                nc.gpsimd.dma_start(out=sk_s, in_=sk_d[:, :, :, :])
                nc.gpsimd.dma_start(out=two_s, in_=two_d[:, :])

                def copy_block(t, qkt, vt):
                    # q/k PSUM -> SBUF bf16 (ACT) frees PSUM fast; v -> vA.
                    # scale 1/WSCALE undoes the host-side fp8 weight scaling.
                    qkb = p1r.tile([128, 2, 512], BF16, tag="qkb")
                    nc.scalar.activation(qkb, qkt, ACT.Copy, scale=1.0 / WSCALE)
                    nc.scalar.activation(
                        vA[:, t, :, 0:64],
                        vt.rearrange("p (h d) -> p h d", d=64),
                        ACT.Copy, scale=1.0 / WSCALE)
                    nc.gpsimd.memset(vA[:, t, :, 64:65], 1.0)
                    return qkb

                def rope_block(t, qkb):
                    for qk in range(2):
                        cos_s = cq_s if qk == 0 else ck_s
                        sin_s = sq_s if qk == 0 else sk_s
                        src = qkb[:, qk, :]
                        qc = p1r.tile([128, 8, 2, 32], BF16, tag=f"qc{qk}")
                        cos_ap = _ap(cos_s, t * 32, [(0, 8), (0, 2), (1, 32)])
                        nc.vector.tensor_tensor(
                            out=qc,
                            in0=src.rearrange("p (h f e) -> p h f e", f=2, e=32),
                            in1=cos_ap, op=AX.mult)
                        sw_ap = _ap(qkb, qk * 512 + 32,
                                    [(64, 8), (-32, 2), (1, 32)])
                        sin_ap = _ap(sin_s, t * 64, [(0, 8), (32, 2), (1, 32)])
                        t2 = p1r.tile([128, 8, 2, 32], BF16, tag=f"t2{qk}")
                        nc.vector.tensor_tensor(out=t2, in0=sw_ap, in1=sin_ap,
                                                op=AX.mult)
                        qf = p1r.tile([128, 512], BF16, tag=f"qf{qk}")
                        nc.vector.tensor_tensor(
                            out=qf.rearrange("p (h f e) -> p h f e", f=2, e=32),
                            in0=qc, in1=t2, op=AX.add)
                        dst = qT if qk == 0 else kT
                        nc.sync.dma_start_transpose(dst[:, :, t, :], qf)

                pending = []
                pending_c = []
                ht_tiles = {}
                for tpre in range(4):
                    ht = p1h.tile([128, H], BF16, tag="ht")
                    nc.scalar.dma_start(out=ht,
                                        in_=hid_d[tpre * 128:(tpre + 1) * 128, :])
                    ht_tiles[tpre] = ht

                for t in range(TOKCH):
                    if t + 4 < TOKCH:
                        htn = p1h.tile([128, H], BF16, tag="ht")
                        nc.scalar.dma_start(
                            out=htn, in_=hid_d[(t + 4) * 128:(t + 5) * 128, :])
                        ht_tiles[t + 4] = htn
                    ht = ht_tiles.pop(t)

                    st6 = p1s.tile([128, 2, 6], F32, tag="st6")
                    for hf in range(2):
                        nc.vector.bn_stats(out=st6[:, hf, :],
                                           in_=ht[:, hf * 512:(hf + 1) * 512])
                    mv = p1s.tile([128, 2], F32, tag="mv")
                    nc.vector.bn_aggr(out=mv, in_=st6)
                    vpe = p1s.tile([128, 1], F32, tag="vpe")
                    nc.gpsimd.tensor_scalar(out=vpe, in0=mv[:, 1:2],
                                            scalar1=LN_EPS, scalar2=None,
                                            op0=AX.add)
                    rstd = p1s.tile([128, 1], F32, tag="rstd")
                    nc.gpsimd.tensor_tensor(out=rstd, in0=vpe, in1=nhalf_s,
                                            op=AX.pow)
                    hn = p1n.tile([128, H], BF16, tag="hn")
                    nc.vector.tensor_scalar(out=hn, in0=ht, scalar1=mv[:, 0:1],
                                            scalar2=rstd, op0=AX.subtract,
                                            op1=AX.mult)

                    # transpose hn on the PE (8 identity matmuls into a bf16
                    # PSUM tile, then one ACT copy to SBUF) -- keeps the
                    # serialized DMA device off the per-chunk critical path.
                    # hn is the stationary operand of the transposes, so fence
                    # with a moving read first (hw ldweights race); the fence
                    # output lands in qkt where the o=0 bias matmul
                    # (start=True) overwrites it.
                    qkt = qkvp.tile([128, 2, 512], F32, tag="qk")
                    vt = vkvp.tile([128, 512], F32, tag="v")
                    tp = tpp.tile([128, 8, 128], BF16, tag="tp")
                    nc.tensor.matmul(qkt[0:1, 0, 0:1], lhsT=onesc_s,
                                     rhs=hn[:, 0:1], start=True, stop=True)
                    for c in range(8):
                        nc.tensor.transpose(tp[:, c, :],
                                            hn[:, c * 128:(c + 1) * 128],
                                            ident_s)
                    # previous chunk's PSUM->SBUF copies enter the ACT queue
                    # FIRST so ACT has work while the PE transposes finish.
                    if pending_c:
                        tc_, qkt_, vt_ = pending_c.pop(0)
                        pending.append((tc_, copy_block(tc_, qkt_, vt_)))

                    # hi/lo fp8 residual split of the transposed activations:
                    # hi = fp8(h) via an ACT copy, lo = fp8(h - hi) on DVE.
                    # (h@w computed as hi@w_hi + hi@w_lo + lo@w_hi with
                    # DoubleRow fp8 matmuls.)
                    hhi = p1t.tile([128, 8, 128], FP8, tag="hhi")
                    nc.scalar.copy(hhi, tp)

                    # rope of chunk t-2 keeps DVE busy during the
                    # transpose->hhi round trip, queued ahead of hlo.
                    if len(pending) > 1:
                        rope_block(*pending.pop(0))

                    hlo = p1t.tile([128, 8, 128], FP8, tag="hlo")
                    nc.vector.tensor_tensor(out=hlo, in0=tp, in1=hhi,
                                            op=AX.subtract)
                    if t == 0:
                        for c in range(2, 8):
                            nc.sync.dma_start(out=whi_s[:, c, :],
                                              in_=whi_d[:, c, :])
                            nc.sync.dma_start(out=wlo_s[:, c, :],
                                              in_=wlo_d[:, c, :])

                    # hi fence (hw ldweights race) then ALL hi-operand
                    # matmuls; the lo fence + lo matmuls go last so the PE has
                    # ~3us of hi work while DVE produces hlo.  o=0/1/2
                    # accumulation groups interleave (different PSUM regions).
                    # v bias is added on the host after the softmax division
                    # (it rides through as b*den/den), so only q/k get the
                    # ones-row bias matmul.
                    def dr_block(dst, a, bmat, o, start):
                        for cp in range(4):
                            nc.tensor.matmul(
                                dst, lhsT=a[:, 2 * cp:2 * cp + 2, :],
                                rhs=bmat[:, 2 * cp:2 * cp + 2,
                                         o * 512:(o + 1) * 512],
                                start=(start and cp == 0), stop=False,
                                perf_mode=mybir.MatmulPerfMode.DoubleRow)

                    # hi fence precedes the q/k group starts; the lo fence
                    # lands in the (not yet started) v region right before
                    # the v group, so neither corrupts an open accumulation.
                    nc.tensor.matmul(qkt[0:1, 0, 0:8], lhsT=onesc_s,
                                     rhs=hhi[:, :, 0:1], start=True, stop=True)
                    for o in range(2):
                        dst = qkt[:, o, :]
                        nc.tensor.matmul(dst, lhsT=onesr_s,
                                         rhs=wb_s[:, o * 512:(o + 1) * 512],
                                         start=True, stop=False)
                        dr_block(dst, hhi, whi_s, o, False)
                        dr_block(dst, hhi, wlo_s, o, False)
                    nc.tensor.matmul(vt[0:1, 0:8], lhsT=onesc_s,
                                     rhs=hlo[:, :, 0:1], start=True, stop=True)
                    dr_block(vt, hhi, whi_s, 2, True)
                    dr_block(vt, hhi, wlo_s, 2, False)
                    for o in range(3):
                        dst = vt if o == 2 else qkt[:, o, :]
                        for cp in range(4):
                            nc.tensor.matmul(
                                dst, lhsT=hlo[:, 2 * cp:2 * cp + 2, :],
                                rhs=whi_s[:, 2 * cp:2 * cp + 2,
                                          o * 512:(o + 1) * 512],
                                start=False, stop=(cp == 3),
                                perf_mode=mybir.MatmulPerfMode.DoubleRow)

                    pending_c.append((t, qkt, vt))
                for tc_, qkt_, vt_ in pending_c:
                    pending.append((tc_, copy_block(tc_, qkt_, vt_)))
                for item in pending:
                    rope_block(*item)

            # ---------------- Phase 2 ----------------
            if "2" not in phases:
                nc.compile()
                return nc
            with tc.tile_pool(name="p2st", bufs=1) as p2st, \
                 tc.tile_pool(name="p2x", bufs=2) as p2x, \
                 tc.tile_pool(name="spp", bufs=3, space="PSUM") as spp, \
                 tc.tile_pool(name="ctxp", bufs=2, space="PSUM") as ctxp:

                outst = p2st.tile([128, HPC, 2, 8, 65], BF16)
                units = [(h, qh) for h in range(HPC) for qh in range(2)]

                def scores_block(ui, kc):
                    h, qh = units[ui]
                    pb = 64 * (h % 2)
                    blk = h // 2
                    Ut = U[ui % 2]
                    sp = spp.tile([128, 1024], F32, tag="sp")
                    for nn in range(2):
                        nc.tensor.matmul(
                            sp[:, nn * 512:(nn + 1) * 512],
                            lhsT=kT[pb:pb + 64, blk, kc, :],
                            rhs=qT[pb:pb + 64, blk,
                                   qh * 8 + nn * 4:qh * 8 + (nn + 1) * 4, :],
                            start=True, stop=True,
                            tile_position=(pb, 0))
                    if kc in CHAIN_KCS or kc == 12:
                        xsb = p2x.tile([128, 1024], F32, tag="xsb")
                        nc.vector.tensor_copy(xsb, sp)
                        two_ap = _ap(two_s, 0, [(0, 512), (1, 2)])
                        nc.gpsimd.tensor_tensor(out=Ut[:, kc, :],
                                                in0=two_ap, in1=xsb,
                                                op=AX.pow)
                    else:
                        nc.scalar.activation(Ut[:, kc, :], sp, ACT.Exp,
                                             scale=LN2)

                def ctx_qchunk(ui, qc):
                    # transposed ctx: out partitions = 128 query tokens,
                    # free = 64 head dims + 1 denominator column.  The tile is
                    # padded to a full 2KB PSUM bank so concurrent accumulation
                    # groups never share a bank.
                    h, qh = units[ui]
                    Ut = U[ui % 2]
                    cpd = ctxp.tile([128, 512], F32, tag="cpd")
                    if qc == 0:
                        # PE fences: on real hardware the weights load
                        # (ldweights) can execute ahead of the matmult's
                        # semaphore wait, so before the first U ldweights of
                        # this unit, read every U kc-slice as the MOVING
                        # operand of throwaway matmuls.  One fence per
                        # producer engine class (Pool chain kcs first, then
                        # all) so each needs a single semaphore wait.
                        nch = 7
                        chain_ap = _ap(Ut, 0, [(1024 * 2, nch), (1, 1)])
                        nc.tensor.matmul(cpd[0:1, 480:480 + nch],
                                         lhsT=onesc_s, rhs=chain_ap,
                                         start=True, stop=True)
                        nc.tensor.matmul(cpd[0:1, 496:512],
                                         lhsT=onesc_s,
                                         rhs=Ut[:, :, 0:1],
                                         start=True, stop=True)
                    for kc in range(TOKCH):
                        nc.tensor.matmul(
                            cpd[:, 0:65], lhsT=Ut[:, kc, qc * 128:(qc + 1) * 128],
                            rhs=vA[:, kc, h, :],
                            start=(kc == 0), stop=(kc == TOKCH - 1))
                    if qc % 2 == 0:
                        nc.vector.tensor_copy(outst[:, h, qh, qc, :],
                                              cpd[:, 0:65])
                    else:
                        nc.scalar.copy(outst[:, h, qh, qc, :], cpd[:, 0:65])

                def head_dma(h):
                    nc.sync.dma_start(out=out_d[h], in_=outst[:, h])

                # ctx of unit ui-1 is interleaved into unit ui's scores from
                # kc=3 on: the sp-pool WAR (bufs=3) forces exp(ui-1,15) to
                # complete before scores(ui,2) executes, so U[ui-1] is fully
                # written before any of its ldweights stream in.
                CTX_SLOTS = {3: 0, 4: 1, 6: 2, 8: 3, 10: 4, 12: 5, 14: 6, 15: 7}
                nu = len(units)
                for ui in range(nu):
                    for kc in range(TOKCH):
                        scores_block(ui, kc)
                        if ui > 0 and kc in CTX_SLOTS:
                            ctx_qchunk(ui - 1, CTX_SLOTS[kc])
                    if ui > 0 and units[ui - 1][1] == 1:
                        head_dma(units[ui - 1][0])
                for qc in range(8):
                    ctx_qchunk(nu - 1, qc)
                head_dma(HPC - 1)

    nc.compile()
    return nc


def _host_inputs(hidden_states, sinusoidal_pos, ln_weight, ln_bias, w_qkv, b_qkv):
    import ml_dtypes
    bf16 = ml_dtypes.bfloat16

    hidden_states = np.ascontiguousarray(hidden_states, dtype=np.float32)
    w_qkv = np.asarray(w_qkv, dtype=np.float32)
    b_qkv = np.asarray(b_qkv, dtype=np.float32)
    ln_weight = np.asarray(ln_weight, dtype=np.float32)
    ln_bias = np.asarray(ln_bias, dtype=np.float32)
    sp = np.asarray(sinusoidal_pos, dtype=np.float32).reshape(S, HD)

    w_eff = ln_weight[:, None] * w_qkv
    b_eff = b_qkv + ln_bias @ w_qkv

    sin_tab = sp[:, :32]
    cos_tab = sp[:, 32:]

    # tables, token-major: [128 tok-in-chunk, chunk, ...]
    cosR = cos_tab.reshape(TOKCH, 128, 32).transpose(1, 0, 2)   # [128, tc, 32]
    sinR = sin_tab.reshape(TOKCH, 128, 32).transpose(1, 0, 2)
    cq = cosR * CEXP
    ck = cosR.copy()
    sq = np.stack([-sinR, sinR], axis=2) * CEXP                 # [128, tc, 2, 32]
    sk = np.stack([-sinR, sinR], axis=2)

    # col order per head: [even d | odd d]
    hh, ff, ee = np.meshgrid(np.arange(8), np.arange(2), np.arange(32),
                             indexing="ij")
    localperm = (hh * 64 + 2 * ee + ff).reshape(-1)             # [512]

    in_maps = []
    for core in range(N_CORES):
        b = core // 2
        g = core % 2
        heads = g * 8 + np.arange(8)
        qcols = heads[localperm // 64] * 64 + localperm % 64
        kcols = 1024 + qcols
        vcols = 2048 + np.repeat(heads, 64) * 64 + np.tile(np.arange(64), 8)
        colsel = np.concatenate([qcols, kcols, vcols])

        wsel = w_eff[:, colsel]                                 # [1024, 1536]
        w3 = wsel.reshape(8, 128, 1536).transpose(1, 0, 2)      # [128, 8, 1536]
        w32 = np.ascontiguousarray(w3) * WSCALE
        fp8 = ml_dtypes.float8_e4m3
        w_hi = w32.astype(fp8)
        w_lo = (w32 - w_hi.astype(np.float32)).astype(fp8)

        in_maps.append({
            "hid": hidden_states[b].astype(bf16),
            "whi": w_hi,
            "wlo": w_lo,
            "wb": (b_eff[colsel][None, :] * WSCALE).astype(bf16),
            "onesr": np.ones((1, 128), np.float32).astype(bf16),
            "cq": np.ascontiguousarray(cq, np.float32).astype(bf16),
            "sq": np.ascontiguousarray(sq, np.float32).astype(bf16),
            "ck": np.ascontiguousarray(ck, np.float32).astype(bf16),
            "sk": np.ascontiguousarray(sk, np.float32).astype(bf16),
            "two": np.full((128, 2), 2.0, np.float32),
        })
    return in_maps


def _run(trace=False, **inputs):
    if "nc" not in _CACHE:
        _CACHE["nc"] = _build_program()
    nc = _CACHE["nc"]
    in_maps = _host_inputs(**inputs)
    res = run_bass_kernel_spmd(nc, in_maps, core_ids=list(range(N_CORES)),
                               trace=trace)

    # v bias is applied here: ctx = sum_k p_k (v_k + b) = num/den + b
    ln_bias = np.asarray(inputs["ln_bias"], dtype=np.float32)
    w_qkv = np.asarray(inputs["w_qkv"], dtype=np.float32)
    b_qkv = np.asarray(inputs["b_qkv"], dtype=np.float32)
    b_eff = b_qkv + ln_bias @ w_qkv
    bias_v = b_eff[2 * H:3 * H]                          # [1024], head-major

    out = np.empty((B, S, H), np.float32)
    for core in range(N_CORES):
        b = core // 2
        g = core % 2
        o = np.asarray(res.results[core]["out"], dtype=np.float32)
        o = o.reshape(HPC, 128, 2, 8, 65)
        blk = o[..., :64] / o[..., 64:65]                # [h, row, qh, qc, d]
        blk = blk.transpose(2, 3, 1, 0, 4).reshape(S, HPC * 64)
        out[b, :, g * 512:(g + 1) * 512] = \
            blk + bias_v[None, g * 512:(g + 1) * 512]
    return out, res


def kernel(**inputs):
    out, _ = _run(trace=False, **inputs)
    return out



# revision 82
# speedup vs baseline: 1.1631x; 1.0044x over previous
"""RoFormer self-attention v4: fp8 DoubleRow QKV + PE transposes +
transposed ctx + multi-engine softmax exp, on 8 trn2 cores.

Sharding: core c -> batch b = c//2, head-group g = c%2 (8 of 16 heads).

Per core:
  Phase 1 (per 128-token chunk t, 16 chunks):
    LN stats (DVE bn_stats) -> hn = (ht-mu)*rstd bf16; ht arrives bf16
    hn -> hT via 8 PE identity-transpose matmuls (bf16 PSUM) -> fp8 hi
      (ACT copy) + fp8 lo = hn - hi (DVE): residual split keeps accuracy
    QKV = hi@w_hi + hi@w_lo + lo@w_hi with fp8 DoubleRow matmuls (2
      k-chunks per instr, 0.5 cyc/col); w_hi/w_lo are host-split fp8 of
      32x-scaled weights, ACT copies scale PSUM by 1/32 on the way out
    RoPE on q/k (DVE): col order per head = [even d | odd d] so the
      rotate-half partner is a contiguous 32-col block; q tables carry
      C = log2(e)/8 so scores come out in exp2 units
    dmaT qf/kf -> qT/kT [128, 4, 16, 128] bf16 (partition 64*(h%2)+d')
    v: PSUM -> vA [128, 16, 8, 65] bf16 + ones column (no bias: v bias is
      added on the host after the softmax division)
  Phase 2 (per unit u = (head h, query-half qh), 16 units):
    scores: 2 bf16 matmuls lhsT=kT [64,128] rhs=qT [64,512] -> sp [128,1024]
    exp: 2^sp -> U[u%2][:, kc, :] bf16; split ACT Exp(scale=ln2) vs
         chain (DVE copy -> Pool pow(2,x)) on CHAIN_KCS
    ctx (transposed): per 128-query chunk, 16 accumulating matmuls
      lhsT=U[keys,q] rhs=vA[keys,65] -> cpd [128 q, 65]; full partition
      use halves the PE cost vs the [65, 512] orientation
    tail: DVE copy -> out staging, one DMA per head.  Division on HOST.

Hardware-race note: on real trn2 the PE weight load (ldweights) does NOT
honor the matmult's semaphore waits, so every freshly-written stationary
operand (hn, hi, lo, U) is preceded by a tiny "fence" matmul that reads
it as the MOVING operand (one fence per producer engine).
"""

import numpy as np

import concourse.bass as bass
import concourse.mybir as mybir
import concourse.tile as tile
from concourse import bacc
from concourse.bass_utils import run_bass_kernel_spmd
from concourse.masks import make_identity

F32 = mybir.dt.float32
BF16 = mybir.dt.bfloat16
FP8 = mybir.dt.float8e4
WSCALE = 32.0
AX = mybir.AluOpType
ACT = mybir.ActivationFunctionType

B, S, H = 4, 2048, 1024
NH, HD = 16, 64
LN_EPS = 1e-12
N_CORES = 8
HPC = 8
TOKCH = 16
CEXP = float(np.log2(np.e) / np.sqrt(HD))
LN2 = float(np.log(2.0))

# chain (DVE-copy + Pool-pow) exp blocks: stride-2 spread over the EARLY
# kcs only -- the chain path has ~2.7us latency after its scores block, so
# the tail kcs must use the fast ACT path or the next unit's U fence
# stalls the PE.  Regular stride keeps the fence AP expressible.
CHAIN_KCS = (0, 2, 4, 6, 8, 10)
import os
FILLERS = int(os.environ.get('FILLERS', '0'))
WARMUPS = int(os.environ.get('WARMUPS', '10'))

_CACHE = {}


def _ap(t, offset_elems, free_dims):
    return bass.AP(tensor=t.tensor, offset=t.offset + offset_elems,
                   ap=[[t.ap[0][0], t.ap[0][1]]] + [[s, n] for s, n in free_dims])


def _build_program(phases="12"):
    nc = bacc.Bacc("TRN2", target_bir_lowering=False)

    hid_d = nc.dram_tensor("hid", [S, H], BF16, kind="ExternalInput")
    whi_d = nc.dram_tensor("whi", [128, 8, 1536], FP8, kind="ExternalInput")
    wlo_d = nc.dram_tensor("wlo", [128, 8, 1536], FP8, kind="ExternalInput")
    wb_d = nc.dram_tensor("wb", [1, 1536], BF16, kind="ExternalInput")
    onesr_d = nc.dram_tensor("onesr", [1, 128], BF16, kind="ExternalInput")
    cq_d = nc.dram_tensor("cq", [128, TOKCH, 32], BF16, kind="ExternalInput")
    sq_d = nc.dram_tensor("sq", [128, TOKCH, 2, 32], BF16, kind="ExternalInput")
    ck_d = nc.dram_tensor("ck", [128, TOKCH, 32], BF16, kind="ExternalInput")
    sk_d = nc.dram_tensor("sk", [128, TOKCH, 2, 32], BF16, kind="ExternalInput")
    two_d = nc.dram_tensor("two", [128, 2], F32, kind="ExternalInput")
    out_d = nc.dram_tensor("out", [HPC, 128, 2, 8, 65], BF16,
                           kind="ExternalOutput")

    with tile.TileContext(nc) as tc:
        with tc.tile_pool(name="const", bufs=1) as const, \
             tc.tile_pool(name="store", bufs=1) as store:
            whi_s = const.tile([128, 8, 1536], FP8)
            wlo_s = const.tile([128, 8, 1536], FP8)
            wb_s = const.tile([1, 1536], BF16)
            onesr_s = const.tile([1, 128], BF16)
            cq_s = const.tile([128, TOKCH, 32], BF16)
            sq_s = const.tile([128, TOKCH, 2, 32], BF16)
            ck_s = const.tile([128, TOKCH, 32], BF16)
            sk_s = const.tile([128, TOKCH, 2, 32], BF16)
            two_s = const.tile([128, 2], F32)
            nhalf_s = const.tile([128, 1], F32)
            onesc_s = const.tile([128, 1], BF16)
            ident_s = const.tile([128, 128], BF16)

            qT = store.tile([128, 4, TOKCH, 128], BF16)
            kT = store.tile([128, 4, TOKCH, 128], BF16)
            vA = store.tile([128, TOKCH, HPC, 65], BF16)
            U0 = store.tile([128, TOKCH, 1024], BF16)
            U1 = store.tile([128, TOKCH, 1024], BF16)
            U = [U0, U1]

            # ---------------- Phase 1 ----------------
            with tc.tile_pool(name="p1h", bufs=4) as p1h, \
                 tc.tile_pool(name="p1s", bufs=4) as p1s, \
                 tc.tile_pool(name="p1n", bufs=2) as p1n, \
                 tc.tile_pool(name="p1t", bufs=4) as p1t, \
                 tc.tile_pool(name="p1r", bufs=4) as p1r, \
                 tc.tile_pool(name="qkvp", bufs=2, space="PSUM") as qkvp, \
                 tc.tile_pool(name="vkvp", bufs=2, space="PSUM") as vkvp, \
                 tc.tile_pool(name="tpp", bufs=2, space="PSUM") as tpp:
                # consts on the gpsimd SWDGE queue, w (chunked) on SP, ht
                # loads on ACT: three queues so the critical chunk-0
                # transpose isn't stuck behind bulk loads.
                nc.sync.dma_start(out=wb_s, in_=wb_d[:, :])
                nc.sync.dma_start(out=onesr_s, in_=onesr_d[:, :])
                nc.gpsimd.memset(nhalf_s, -0.5)
                nc.gpsimd.memset(onesc_s, 1.0)
                make_identity(nc, ident_s)
                wu = qkvp.tile([128, 2, 512], F32, tag="qk")
                for _ in range(WARMUPS):
                    nc.tensor.matmul(wu[:, 0, :], lhsT=onesr_s,
                                     rhs=wb_s[:, 0:512], start=True, stop=True)
                # only the first two w chunks load ahead of the chunk-0
                # transpose; the rest are issued after it (see chunk loop) so
                # the serialized DMA device reaches hTb(0) early.
                for c in range(2):
                    nc.sync.dma_start(out=whi_s[:, c, :], in_=whi_d[:, c, :])
                    nc.sync.dma_start(out=wlo_s[:, c, :], in_=wlo_d[:, c, :])
                nc.gpsimd.dma_start(out=cq_s, in_=cq_d[:, :, :])
                nc.gpsimd.dma_start(out=sq_s, in_=sq_d[:, :, :, :])
                nc.gpsimd.dma_start(out=ck_s, in_=ck_d[:, :, :])
                nc.gpsimd.dma_start(out=sk_s, in_=sk_d[:, :, :, :])
                nc.gpsimd.dma_start(out=two_s, in_=two_d[:, :])

                def copy_block(t, qkt, vt):
                    # q/k PSUM -> SBUF bf16 (ACT) frees PSUM fast; v -> vA.
                    # scale 1/WSCALE undoes the host-side fp8 weight scaling.
                    qkb = p1r.tile([128, 2, 512], BF16, tag="qkb")
                    nc.scalar.activation(qkb, qkt, ACT.Copy, scale=1.0 / WSCALE)
                    nc.scalar.activation(
                        vA[:, t, :, 0:64],
                        vt.rearrange("p (h d) -> p h d", d=64),
                        ACT.Copy, scale=1.0 / WSCALE)
                    nc.gpsimd.memset(vA[:, t, :, 64:65], 1.0)
                    return qkb

                def rope_block(t, qkb):
                    for qk in range(2):
                        cos_s = cq_s if qk == 0 else ck_s
                        sin_s = sq_s if qk == 0 else sk_s
                        src = qkb[:, qk, :]
                        qc = p1r.tile([128, 8, 2, 32], BF16, tag=f"qc{qk}")
                        cos_ap = _ap(cos_s, t * 32, [(0, 8), (0, 2), (1, 32)])
                        nc.vector.tensor_tensor(
                            out=qc,
                            in0=src.rearrange("p (h f e) -> p h f e", f=2, e=32),
                            in1=cos_ap, op=AX.mult)
                        sw_ap = _ap(qkb, qk * 512 + 32,
                                    [(64, 8), (-32, 2), (1, 32)])
                        sin_ap = _ap(sin_s, t * 64, [(0, 8), (32, 2), (1, 32)])
                        t2 = p1r.tile([128, 8, 2, 32], BF16, tag=f"t2{qk}")
                        nc.vector.tensor_tensor(out=t2, in0=sw_ap, in1=sin_ap,
                                                op=AX.mult)
                        qf = p1r.tile([128, 512], BF16, tag=f"qf{qk}")
                        nc.vector.tensor_tensor(
                            out=qf.rearrange("p (h f e) -> p h f e", f=2, e=32),
                            in0=qc, in1=t2, op=AX.add)
                        dst = qT if qk == 0 else kT
                        nc.sync.dma_start_transpose(dst[:, :, t, :], qf)

                pending = []
                pending_c = []
                ht_tiles = {}
                for tpre in range(4):
                    ht = p1h.tile([128, H], BF16, tag="ht")
                    nc.scalar.dma_start(out=ht,
                                        in_=hid_d[tpre * 128:(tpre + 1) * 128, :])
                    ht_tiles[tpre] = ht

                for t in range(TOKCH):
                    if t + 4 < TOKCH:
                        htn = p1h.tile([128, H], BF16, tag="ht")
                        nc.scalar.dma_start(
                            out=htn, in_=hid_d[(t + 4) * 128:(t + 5) * 128, :])
                        ht_tiles[t + 4] = htn
                    ht = ht_tiles.pop(t)

                    st6 = p1s.tile([128, 2, 6], F32, tag="st6")
                    for hf in range(2):
                        nc.vector.bn_stats(out=st6[:, hf, :],
                                           in_=ht[:, hf * 512:(hf + 1) * 512])
                    mv = p1s.tile([128, 2], F32, tag="mv")
                    nc.vector.bn_aggr(out=mv, in_=st6)
                    vpe = p1s.tile([128, 1], F32, tag="vpe")
                    nc.gpsimd.tensor_scalar(out=vpe, in0=mv[:, 1:2],
                                            scalar1=LN_EPS, scalar2=None,
                                            op0=AX.add)
                    rstd = p1s.tile([128, 1], F32, tag="rstd")
                    nc.gpsimd.tensor_tensor(out=rstd, in0=vpe, in1=nhalf_s,
                                            op=AX.pow)
                    hn = p1n.tile([128, H], BF16, tag="hn")
                    nc.vector.tensor_scalar(out=hn, in0=ht, scalar1=mv[:, 0:1],
                                            scalar2=rstd, op0=AX.subtract,
                                            op1=AX.mult)

                    # transpose hn on the PE (8 identity matmuls into a bf16
                    # PSUM tile, then one ACT copy to SBUF) -- keeps the
                    # serialized DMA device off the per-chunk critical path.
                    # hn is the stationary operand of the transposes, so fence
                    # with a moving read first (hw ldweights race); the fence
                    # output lands in qkt where the o=0 bias matmul
                    # (start=True) overwrites it.
                    qkt = qkvp.tile([128, 2, 512], F32, tag="qk")
                    vt = vkvp.tile([128, 512], F32, tag="v")
                    tp = tpp.tile([128, 8, 128], BF16, tag="tp")
                    nc.tensor.matmul(qkt[0:1, 0, 0:1], lhsT=onesc_s,
                                     rhs=hn[:, 0:1], start=True, stop=True)
                    for c in range(8):
                        nc.tensor.transpose(tp[:, c, :],
                                            hn[:, c * 128:(c + 1) * 128],
                                            ident_s)
                    # previous chunk's PSUM->SBUF copies enter the ACT queue
                    # FIRST so ACT has work while the PE transposes finish.
                    if pending_c:
                        tc_, qkt_, vt_ = pending_c.pop(0)
                        pending.append((tc_, copy_block(tc_, qkt_, vt_)))

                    # hi/lo fp8 residual split of the transposed activations:
                    # hi = fp8(h) via an ACT copy, lo = fp8(h - hi) on DVE.
                    # (h@w computed as hi@w_hi + hi@w_lo + lo@w_hi with
                    # DoubleRow fp8 matmuls.)
                    hhi = p1t.tile([128, 8, 128], FP8, tag="hhi")
                    nc.scalar.copy(hhi, tp)

                    # rope of chunk t-2 keeps DVE busy during the
                    # transpose->hhi round trip, queued ahead of hlo.
                    if len(pending) > 1:
                        rope_block(*pending.pop(0))

                    hlo = p1t.tile([128, 8, 128], FP8, tag="hlo")
                    nc.vector.tensor_tensor(out=hlo, in0=tp, in1=hhi,
                                            op=AX.subtract)
                    if t == 0:
                        for c in range(2, 8):
                            nc.sync.dma_start(out=whi_s[:, c, :],
                                              in_=whi_d[:, c, :])
                            nc.sync.dma_start(out=wlo_s[:, c, :],
                                              in_=wlo_d[:, c, :])

                    # hi fence (hw ldweights race) then ALL hi-operand
                    # matmuls; the lo fence + lo matmuls go last so the PE has
                    # ~3us of hi work while DVE produces hlo.  o=0/1/2
                    # accumulation groups interleave (different PSUM regions).
                    # v bias is added on the host after the softmax division
                    # (it rides through as b*den/den), so only q/k get the
                    # ones-row bias matmul.
                    def dr_block(dst, a, bmat, o, start):
                        for cp in range(4):
                            nc.tensor.matmul(
                                dst, lhsT=a[:, 2 * cp:2 * cp + 2, :],
                                rhs=bmat[:, 2 * cp:2 * cp + 2,
                                         o * 512:(o + 1) * 512],
                                start=(start and cp == 0), stop=False,
                                perf_mode=mybir.MatmulPerfMode.DoubleRow)

                    # hi fence precedes the q/k group starts; the lo fence
                    # lands in the (not yet started) v region right before
                    # the v group, so neither corrupts an open accumulation.
                    nc.tensor.matmul(qkt[0:1, 0, 0:8], lhsT=onesc_s,
                                     rhs=hhi[:, :, 0:1], start=True, stop=True)
                    for o in range(2):
                        dst = qkt[:, o, :]
                        nc.tensor.matmul(dst, lhsT=onesr_s,
                                         rhs=wb_s[:, o * 512:(o + 1) * 512],
                                         start=True, stop=False)
                        dr_block(dst, hhi, whi_s, o, False)
                        dr_block(dst, hhi, wlo_s, o, False)
                    nc.tensor.matmul(vt[0:1, 0:8], lhsT=onesc_s,
                                     rhs=hlo[:, :, 0:1], start=True, stop=True)
                    dr_block(vt, hhi, whi_s, 2, True)
                    dr_block(vt, hhi, wlo_s, 2, False)
                    for o in range(3):
                        dst = vt if o == 2 else qkt[:, o, :]
                        for cp in range(4):
                            nc.tensor.matmul(
                                dst, lhsT=hlo[:, 2 * cp:2 * cp + 2, :],
                                rhs=whi_s[:, 2 * cp:2 * cp + 2,
                                          o * 512:(o + 1) * 512],
                                start=False, stop=(cp == 3),
                                perf_mode=mybir.MatmulPerfMode.DoubleRow)

                    pending_c.append((t, qkt, vt))
                for tc_, qkt_, vt_ in pending_c:
                    pending.append((tc_, copy_block(tc_, qkt_, vt_)))
                for item in pending:
                    rope_block(*item)

            # ---------------- Phase 2 ----------------
            if "2" not in phases:
                nc.compile()
                return nc
            with tc.tile_pool(name="p2st", bufs=1) as p2st, \
                 tc.tile_pool(name="p2x", bufs=4) as p2x, \
                 tc.tile_pool(name="spp", bufs=3, space="PSUM") as spp, \
                 tc.tile_pool(name="ctxp", bufs=2, space="PSUM") as ctxp:

                outst = p2st.tile([128, HPC, 2, 8, 65], BF16)
                units = [(h, qh) for h in range(HPC) for qh in range(2)]

                def scores_block(ui, kc):
                    h, qh = units[ui]
                    pb = 64 * (h % 2)
                    blk = h // 2
                    Ut = U[ui % 2]
                    sp = spp.tile([128, 1024], F32, tag="sp")
                    for nn in range(2):
                        nc.tensor.matmul(
                            sp[:, nn * 512:(nn + 1) * 512],
                            lhsT=kT[pb:pb + 64, blk, kc, :],
                            rhs=qT[pb:pb + 64, blk,
                                   qh * 8 + nn * 4:qh * 8 + (nn + 1) * 4, :],
                            start=True, stop=True,
                            tile_position=(pb, 0))
                    if kc in CHAIN_KCS or kc == 12:
                        xsb = p2x.tile([128, 1024], F32, tag="xsb")
                        nc.vector.tensor_copy(xsb, sp)
                        two_ap = _ap(two_s, 0, [(0, 512), (1, 2)])
                        nc.gpsimd.tensor_tensor(out=Ut[:, kc, :],
                                                in0=two_ap, in1=xsb,
                                                op=AX.pow)
                    else:
                        nc.scalar.activation(Ut[:, kc, :], sp, ACT.Exp,
                                             scale=LN2)

                def ctx_qchunk(ui, qc):
                    # transposed ctx: out partitions = 128 query tokens,
                    # free = 64 head dims + 1 denominator column.  The tile is
                    # padded to a full 2KB PSUM bank so concurrent accumulation
                    # groups never share a bank.
                    h, qh = units[ui]
                    Ut = U[ui % 2]
                    cpd = ctxp.tile([128, 512], F32, tag="cpd")
                    if qc == 0:
                        # PE fences: on real hardware the weights load
                        # (ldweights) can execute ahead of the matmult's
                        # semaphore wait, so before the first U ldweights of
                        # this unit, read every U kc-slice as the MOVING
                        # operand of throwaway matmuls.  One fence per
                        # producer engine class (Pool chain kcs first, then
                        # all) so each needs a single semaphore wait.
                        nch = 7
                        chain_ap = _ap(Ut, 0, [(1024 * 2, nch), (1, 1)])
                        nc.tensor.matmul(cpd[0:1, 480:480 + nch],
                                         lhsT=onesc_s, rhs=chain_ap,
                                         start=True, stop=True)
                        nc.tensor.matmul(cpd[0:1, 496:512],
                                         lhsT=onesc_s,
                                         rhs=Ut[:, :, 0:1],
                                         start=True, stop=True)
                    for kc in range(TOKCH):
                        nc.tensor.matmul(
                            cpd[:, 0:65], lhsT=Ut[:, kc, qc * 128:(qc + 1) * 128],
                            rhs=vA[:, kc, h, :],
                            start=(kc == 0), stop=(kc == TOKCH - 1))
                    if qc % 8 != 7:
                        nc.vector.tensor_copy(outst[:, h, qh, qc, :],
                                              cpd[:, 0:65])
                    else:
                        nc.scalar.copy(outst[:, h, qh, qc, :], cpd[:, 0:65])

                def head_dma(h):
                    nc.sync.dma_start(out=out_d[h], in_=outst[:, h])

                # ctx of unit ui-1 is interleaved into unit ui's scores from
                # kc=3 on: the sp-pool WAR (bufs=3) forces exp(ui-1,15) to
                # complete before scores(ui,2) executes, so U[ui-1] is fully
                # written before any of its ldweights stream in.
                CTX_SLOTS = {3: 0, 4: 1, 6: 2, 8: 3, 10: 4, 12: 5, 14: 6, 15: 7}
                nu = len(units)
                for ui in range(nu):
                    for kc in range(TOKCH):
                        scores_block(ui, kc)
                        if ui > 0 and kc in CTX_SLOTS:
                            ctx_qchunk(ui - 1, CTX_SLOTS[kc])
                    if ui > 0 and units[ui - 1][1] == 1:
                        head_dma(units[ui - 1][0])
                for qc in range(8):
                    ctx_qchunk(nu - 1, qc)
                head_dma(HPC - 1)

    nc.compile()
    return nc


def _host_inputs(hidden_states, sinusoidal_pos, ln_weight, ln_bias, w_qkv, b_qkv):
    import ml_dtypes
    bf16 = ml_dtypes.bfloat16

    hidden_states = np.ascontiguousarray(hidden_states, dtype=np.float32)
    w_qkv = np.asarray(w_qkv, dtype=np.float32)
    b_qkv = np.asarray(b_qkv, dtype=np.float32)
    ln_weight = np.asarray(ln_weight, dtype=np.float32)
    ln_bias = np.asarray(ln_bias, dtype=np.float32)
    sp = np.asarray(sinusoidal_pos, dtype=np.float32).reshape(S, HD)

    w_eff = ln_weight[:, None] * w_qkv
    b_eff = b_qkv + ln_bias @ w_qkv

    sin_tab = sp[:, :32]
    cos_tab = sp[:, 32:]

    # tables, token-major: [128 tok-in-chunk, chunk, ...]
    cosR = cos_tab.reshape(TOKCH, 128, 32).transpose(1, 0, 2)   # [128, tc, 32]
    sinR = sin_tab.reshape(TOKCH, 128, 32).transpose(1, 0, 2)
    cq = cosR * CEXP
    ck = cosR.copy()
    sq = np.stack([-sinR, sinR], axis=2) * CEXP                 # [128, tc, 2, 32]
    sk = np.stack([-sinR, sinR], axis=2)

    # col order per head: [even d | odd d]
    hh, ff, ee = np.meshgrid(np.arange(8), np.arange(2), np.arange(32),
                             indexing="ij")
    localperm = (hh * 64 + 2 * ee + ff).reshape(-1)             # [512]

    in_maps = []
    for core in range(N_CORES):
        b = core // 2
        g = core % 2
        heads = g * 8 + np.arange(8)
        qcols = heads[localperm // 64] * 64 + localperm % 64
        kcols = 1024 + qcols
        vcols = 2048 + np.repeat(heads, 64) * 64 + np.tile(np.arange(64), 8)
        colsel = np.concatenate([qcols, kcols, vcols])

        wsel = w_eff[:, colsel]                                 # [1024, 1536]
        w3 = wsel.reshape(8, 128, 1536).transpose(1, 0, 2)      # [128, 8, 1536]
        w32 = np.ascontiguousarray(w3) * WSCALE
        fp8 = ml_dtypes.float8_e4m3
        w_hi = w32.astype(fp8)
        w_lo = (w32 - w_hi.astype(np.float32)).astype(fp8)

        in_maps.append({
            "hid": hidden_states[b].astype(bf16),
            "whi": w_hi,
            "wlo": w_lo,
            "wb": (b_eff[colsel][None, :] * WSCALE).astype(bf16),
            "onesr": np.ones((1, 128), np.float32).astype(bf16),
            "cq": np.ascontiguousarray(cq, np.float32).astype(bf16),
            "sq": np.ascontiguousarray(sq, np.float32).astype(bf16),
            "ck": np.ascontiguousarray(ck, np.float32).astype(bf16),
            "sk": np.ascontiguousarray(sk, np.float32).astype(bf16),
            "two": np.full((128, 2), 2.0, np.float32),
        })
    return in_maps


def _run(trace=False, **inputs):
    if "nc" not in _CACHE:
        _CACHE["nc"] = _build_program()
    nc = _CACHE["nc"]
    in_maps = _host_inputs(**inputs)
    res = run_bass_kernel_spmd(nc, in_maps, core_ids=list(range(N_CORES)),
                               trace=trace)

    # v bias is applied here: ctx = sum_k p_k (v_k + b) = num/den + b
    ln_bias = np.asarray(inputs["ln_bias"], dtype=np.float32)
    w_qkv = np.asarray(inputs["w_qkv"], dtype=np.float32)
    b_qkv = np.asarray(inputs["b_qkv"], dtype=np.float32)
    b_eff = b_qkv + ln_bias @ w_qkv
    bias_v = b_eff[2 * H:3 * H]                          # [1024], head-major

    out = np.empty((B, S, H), np.float32)
    for core in range(N_CORES):
        b = core // 2
        g = core % 2
        o = np.asarray(res.results[core]["out"], dtype=np.float32)
        o = o.reshape(HPC, 128, 2, 8, 65)
        blk = o[..., :64] / o[..., 64:65]                # [h, row, qh, qc, d]
        blk = blk.transpose(2, 3, 1, 0, 4).reshape(S, HPC * 64)
        out[b, :, g * 512:(g + 1) * 512] = \
            blk + bias_v[None, g * 512:(g + 1) * 512]
    return out, res


def kernel(**inputs):
    out, _ = _run(trace=False, **inputs)
    return out

